# revision 1
# baseline (speedup 1.0000x reference)
"""Trainium2 Bass kernel for a pre-norm transformer block (B=1, T=4096, C=1024, H=16).

Sharding (8 cores): head-tensor-parallel attention (2 heads/core) with
sequence-parallel LayerNorms and sequence-local (data-parallel) MLP.
All activations are kept transposed on-chip ([C, T] with C on partitions)
so every matmul contracts over the partition axis with K=128 chunks.

Per core c (owns T-columns [512c, 512c+512) and heads 2c, 2c+1):
  1. LN1 on own xT columns -> h1T_c [1024, 512]
  2. AllGather h1T -> h1all [8192, 512] (full h1T, column-blocked by rank)
  3. qkvT = wqkv_c.T @ h1T_full for the 2 local heads  (full T)
  4. causal flash attention (no max subtraction; scores are O(1) here)
  5. AllToAll head-outputs -> aoutT rows-complete for own T columns
  6. x2T = xT + wproj.T @ aoutT (own columns)
  7. LN2 -> h2T; MLP (full weights, own columns); out = x2T + mlpT

Matmul inputs use float32r (full-rate PE) except the PV stage (bf16).
"""
import numpy as np
import ml_dtypes

import concourse.bass as bass
import concourse.bacc as bacc
import concourse.tile as tile
import concourse.mybir as mybir
from concourse import bass_utils

F32 = mybir.dt.float32
F32R = mybir.dt.float32r
BF16 = mybir.dt.bfloat16
AF = mybir.ActivationFunctionType
OP = mybir.AluOpType

NCORES = 8
C = 1024
T = 4096
TC = T // NCORES          # 512 own T columns
CK = C // 128             # 8 C chunks
HS = 64
NQT = TC // 512 * 8       # 8 q-tiles of 512 over full T
FC = 4096                 # MLP hidden
EPS = 1e-5

_CACHE = {}
DEBUG = False


def _layer_norm(nc, tc, sb, x_t, w_ap, out_t, eps_t, ones_t, tag):
    """x_t: [128, 4096] F32R tile (chunk k at cols 512k), w_ap [128,8],
    out_t [128, 4096] F32R. LN over the C axis (partitions x chunks)."""
    with tc.tile_pool(name=f"ln{tag}", bufs=1) as lnp, \
         tc.tile_pool(name=f"lnps{tag}", bufs=1, space="PSUM") as lps:
        mean_ps = lps.tile([128, 512], F32, name=f"mean{tag}")
        sq_ps = lps.tile([128, 512], F32, name=f"sqs{tag}")
        sq_tiles = []
        for k in range(CK):
            sq = lnp.tile([128, 512], F32R, name=f"sq{tag}", bufs=2)
            nc.scalar.activation(sq[:], x_t[:, 512 * k:512 * (k + 1)].bitcast(F32),
                                 AF.Square)
            sq_tiles.append(sq)
        for k in range(CK):
            nc.tensor.matmul(mean_ps[:], ones_t[:],
                             x_t[:, 512 * k:512 * (k + 1)],
                             start=(k == 0), stop=(k == CK - 1))
        for k in range(CK):
            nc.tensor.matmul(sq_ps[:], ones_t[:], sq_tiles[k][:],
                             start=(k == 0), stop=(k == CK - 1))
        # mu and rstd, broadcast along partitions already (ones-matmul)
        mu = lnp.tile([128, 512], F32, name=f"mu{tag}")
        nc.vector.tensor_scalar_mul(mu[:], mean_ps[:], 1.0 / C)
        musq = lnp.tile([128, 512], F32, name=f"musq{tag}")
        nc.vector.tensor_mul(musq[:], mu[:], mu[:])
        var = lnp.tile([128, 512], F32, name=f"var{tag}")
        nc.vector.scalar_tensor_tensor(var[:], sq_ps[:], 1.0 / C, musq[:],
                                       OP.mult, OP.subtract)
        lnv = lnp.tile([128, 512], F32, name=f"lnv{tag}")
        nc.scalar.activation(lnv[:], var[:], AF.Ln, bias=eps_t[:])
        rstd = lnp.tile([128, 512], F32, name=f"rstd{tag}")
        nc.scalar.activation(rstd[:], lnv[:], AF.Exp, scale=-0.5)
        for k in range(CK):
            d = lnp.tile([128, 512], F32, name=f"d{tag}", bufs=2)
            nc.vector.tensor_sub(d[:], x_t[:, 512 * k:512 * (k + 1)].bitcast(F32),
                                 mu[:])
            nc.vector.scalar_tensor_tensor(
                out_t[:, 512 * k:512 * (k + 1)], d[:], w_ap[:, k:k + 1],
                rstd[:], OP.mult, OP.mult)


def _build():
    nc = bacc.Bacc("TRN2", target_bir_lowering=False, debug=False,
                   enable_asserts=False, num_devices=NCORES)

    xT = nc.dram_tensor("xT", [C, TC], F32, kind="ExternalInput").ap()
    wqkv = nc.dram_tensor("wqkv", [C, 3 * 128], F32, kind="ExternalInput").ap()
    wproj = nc.dram_tensor("wproj", [C, C], F32, kind="ExternalInput").ap()
    wfc = nc.dram_tensor("wfc", [C, FC], F32, kind="ExternalInput").ap()
    wmlp = nc.dram_tensor("wmlp", [FC, C], F32, kind="ExternalInput").ap()
    ln1w = nc.dram_tensor("ln1w", [128, CK], F32, kind="ExternalInput").ap()
    ln2w = nc.dram_tensor("ln2w", [128, CK], F32, kind="ExternalInput").ap()
    masks = nc.dram_tensor("masks", [128, 4 * 512], BF16, kind="ExternalInput").ap()
    ident = nc.dram_tensor("ident", [128, 128], F32, kind="ExternalInput").ap()
    onesw = nc.dram_tensor("onesw", [128, 128], F32, kind="ExternalInput").ap()
    zeros = nc.dram_tensor("zeros", [64, T], F32, kind="ExternalInput").ap()
    outT = nc.dram_tensor("outT", [C, TC], F32, kind="ExternalOutput").ap()
    dbg = {}
    if DEBUG:
        for nm in ("d_h1T", "d_qp0", "d_qp1", "d_kT", "d_vT", "d_oT",
                   "d_x2T", "d_h2T"):
            dbg[nm] = nc.dram_tensor(nm, [128, T], F32,
                                     kind="ExternalOutput").ap()
        dbg["d_rl"] = nc.dram_tensor("d_rl", [128, 8], F32,
                                     kind="ExternalOutput").ap()
        dbg["d_est"] = nc.dram_tensor("d_est", [128, 512], F32,
                                      kind="ExternalOutput").ap()

    rg = [list(range(NCORES))]

    with tile.TileContext(nc) as tc:
        with tc.tile_pool(name="dram", bufs=1, space="DRAM") as dramp:
            h1b = dramp.tile([C, TC], F32, name="h1b")
            h1all = dramp.tile([NCORES * C, TC], F32, name="h1all",
                               addr_space="Shared")
            ob = dramp.tile([C, TC], F32, name="ob")
            oax = dramp.tile([C, TC], F32, name="oax")

            with tc.tile_pool(name="glob", bufs=1) as gp:
                ident_t = gp.tile([128, 128], F32, name="ident_t")
                nc.sync.dma_start(ident_t[:], ident[:])
                ones_t = gp.tile([128, 128], F32R, name="ones_t")
                nc.sync.dma_start(ones_t[:], onesw[:].bitcast(F32R))
                ln1w_t = gp.tile([128, CK], F32, name="ln1w_t")
                nc.sync.dma_start(ln1w_t[:], ln1w[:])
                ln2w_t = gp.tile([128, CK], F32, name="ln2w_t")
                nc.sync.dma_start(ln2w_t[:], ln2w[:])
                masks_t = gp.tile([128, 4 * 512], BF16, name="masks_t")
                nc.sync.dma_start(masks_t[:], masks[:])
                eps_t = gp.tile([128, 1], F32, name="eps_t")
                nc.vector.memset(eps_t[:], EPS)
                xT_t = gp.tile([128, T], F32R, name="xT_t")
                for k in range(CK):
                    nc.sync.dma_start(xT_t[:, 512 * k:512 * (k + 1)],
                                      xT[128 * k:128 * (k + 1), :].bitcast(F32R))
                x2T_t = gp.tile([128, T], F32R, name="x2T_t")
                outT_sb = gp.tile([128, T], F32, name="outT_sb")

                # ---------------- attention scope ----------------
                with tc.tile_pool(name="attn", bufs=1) as ap:
                    h1T_t = ap.tile([128, T], F32R, name="h1T_t")
                    _layer_norm(nc, tc, ap, xT_t, ln1w_t, h1T_t, eps_t,
                                ones_t, "1")
                    for k in range(CK):
                        nc.sync.dma_start(h1b[128 * k:128 * (k + 1), :],
                                          h1T_t[:, 512 * k:512 * (k + 1)].bitcast(F32))
                    nc.gpsimd.collective_compute(
                        "AllGather", OP.bypass, replica_groups=rg,
                        ins=[h1b.opt()], outs=[h1all.opt()])

                    # persistent attention tensors
                    qp0 = ap.tile([128, T], F32R, name="qp0")
                    qp1 = ap.tile([128, T], F32R, name="qp1")
                    kT_t = ap.tile([128, T], F32R, name="kT_t")
                    vT_t = ap.tile([128, T], F32, name="vT_t")
                    oT_t = ap.tile([128, T], F32, name="oT_t")
                    nc.sync.dma_start(qp0[64:128, :], zeros[:].bitcast(F32R))
                    nc.sync.dma_start(qp1[0:64, :], zeros[:].bitcast(F32R))

                    # qkvT: for each T-block j, accumulate over C chunks
                    wq_tiles = []
                    for k in range(CK):
                        wq = ap.tile([128, 384], F32R, name=f"wq{k}")
                        nc.sync.dma_start(wq[:], wqkv[128 * k:128 * (k + 1), :]
                                          .bitcast(F32R))
                        wq_tiles.append(wq)
                    with tc.tile_pool(name="qkvs", bufs=1) as qs, \
                         tc.tile_pool(name="qkvps", bufs=1, space="PSUM") as qps:
                        for j in range(NCORES):
                            h1rt = []
                            for k in range(CK):
                                hr = qs.tile([128, 512], F32R, name="h1rt", bufs=10)
                                nc.sync.dma_start(
                                    hr[:],
                                    h1all[C * j + 128 * k:C * j + 128 * (k + 1), :]
                                    .bitcast(F32R))
                                h1rt.append(hr)
                            for m in range(3):
                                pm = qps.tile([128, 512], F32, name="qkvp", bufs=3)
                                for k in range(CK):
                                    nc.tensor.matmul(
                                        pm[:], wq_tiles[k][:, 128 * m:128 * (m + 1)],
                                        h1rt[k][:], start=(k == 0),
                                        stop=(k == CK - 1))
                                blk = slice(512 * j, 512 * (j + 1))
                                if m == 0:
                                    nc.vector.tensor_copy(qp0[0:64, blk], pm[0:64, :])
                                    nc.vector.tensor_copy(qp1[64:128, blk],
                                                          pm[64:128, :])
                                elif m == 1:
                                    nc.vector.tensor_copy(kT_t[:, blk], pm[:])
                                else:
                                    nc.vector.tensor_copy(vT_t[:, blk], pm[:])

                    # v_ext: transpose vT into per-head [T,64]+ones tiles (bf16)
                    ve = [[], []]
                    with tc.tile_pool(name="veps", bufs=1, space="PSUM") as vps:
                        for t in range(T // 128):
                            tp = vps.tile([128, 128], F32, name="vtp", bufs=2)
                            nc.tensor.transpose(tp[:],
                                                vT_t[:, 128 * t:128 * (t + 1)],
                                                ident_t[:])
                            for h in range(2):
                                vx = ap.tile([128, 65], BF16, name=f"ve{h}_{t}")
                                nc.scalar.activation(vx[:, 0:64],
                                                     tp[:, 64 * h:64 * (h + 1)],
                                                     AF.Copy)
                                nc.vector.memset(vx[:, 64:65], 1.0)
                                ve[h].append(vx)

                    # flash attention (no max subtraction), 2 heads
                    qp = [qp0, qp1]
                    with tc.tile_pool(name="atw", bufs=1) as aw, \
                         tc.tile_pool(name="atps", bufs=1, space="PSUM") as aps:
                        dbg_est_sb = aw.tile([128, 512], F32, name="dbg_est") \
                            if DEBUG else None
                        dbg_rl_sb = aw.tile([128, 8], F32, name="dbg_rl") \
                            if DEBUG else None
                        for qi in range(NQT):
                            nkb = 4 * (qi + 1)
                            o2_tiles = [aw.tile([128, 128], F32, name="o2",
                                                bufs=6) for _ in range(4)]
                            for h in range(2):
                                # one PSUM bank per q-subtile accumulator:
                                # start=True resets the whole bank, so groups
                                # must not share banks
                                ops_qs = [aps.tile([128, 65], F32,
                                                   name=f"opsq{q}", bufs=1)
                                          for q in range(4)]
                                for kb in range(nkb):
                                    st = aps.tile([128, 512], F32, name="st",
                                                  bufs=2)
                                    nc.tensor.matmul(
                                        st[:], kT_t[:, 128 * kb:128 * (kb + 1)],
                                        qp[h][:, 512 * qi:512 * (qi + 1)],
                                        start=True, stop=True)
                                    est = aw.tile([128, 512], BF16, name="est",
                                                  bufs=4)
                                    nc.scalar.activation(est[:], st[:], AF.Exp,
                                                         scale=0.125)
                                    if kb >= 4 * qi:
                                        jm = kb - 4 * qi
                                        nc.vector.tensor_mul(
                                            est[:], est[:],
                                            masks_t[:, 512 * jm:512 * (jm + 1)])
                                    if DEBUG and qi == 0 and h == 0:
                                        nc.vector.tensor_copy(
                                            dbg_est_sb[:, 128 * kb:128 * (kb + 1)],
                                            est[:, 0:128])
                                    for qs_i in range(4):
                                        nc.tensor.matmul(
                                            ops_qs[qs_i][:],
                                            est[:, 128 * qs_i:128 * (qs_i + 1)],
                                            ve[h][kb][:],
                                            start=(kb == 0), stop=(kb == nkb - 1))
                                for qs_i in range(4):
                                    lt = aw.tile([128, 1], F32, name="lt", bufs=4)
                                    nc.scalar.activation(lt[:],
                                                         ops_qs[qs_i][:, 64:65],
                                                         AF.Ln)
                                    rl = aw.tile([128, 1], F32, name="rl", bufs=4)
                                    nc.scalar.activation(rl[:], lt[:], AF.Exp,
                                                         scale=-1.0)
                                    nc.vector.tensor_scalar_mul(
                                        o2_tiles[qs_i][:, 64 * h:64 * (h + 1)],
                                        ops_qs[qs_i][:, 0:64], rl[:])
                                    if DEBUG and qi == 0 and h == 0:
                                        nc.vector.tensor_copy(
                                            dbg_rl_sb[:, qs_i:qs_i + 1], rl[:])
                            for qs_i in range(4):
                                tps = aps.tile([128, 128], F32, name="tps", bufs=2)
                                nc.tensor.transpose(tps[:], o2_tiles[qs_i][:],
                                                    ident_t[:])
                                col = 512 * qi + 128 * qs_i
                                nc.scalar.activation(oT_t[:, col:col + 128],
                                                     tps[:], AF.Copy)

                    if DEBUG:
                        nc.sync.dma_start(dbg["d_h1T"][:], h1T_t[:].bitcast(F32))
                        nc.sync.dma_start(dbg["d_qp0"][:], qp0[:].bitcast(F32))
                        nc.sync.dma_start(dbg["d_qp1"][:], qp1[:].bitcast(F32))
                        nc.sync.dma_start(dbg["d_kT"][:], kT_t[:].bitcast(F32))
                        nc.sync.dma_start(dbg["d_vT"][:], vT_t[:])
                        nc.sync.dma_start(dbg["d_oT"][:], oT_t[:])

                    if DEBUG:
                        nc.sync.dma_start(dbg["d_rl"][:], dbg_rl_sb[:])

                    # exchange head outputs: AllToAll
                    for j in range(NCORES):
                        nc.sync.dma_start(ob[128 * j:128 * (j + 1), :],
                                          oT_t[:, 512 * j:512 * (j + 1)])
                    nc.gpsimd.collective_compute(
                        "AllToAll", OP.bypass, replica_groups=rg,
                        ins=[ob.opt()], outs=[oax.opt()])

                    # proj: x2T = xT + wproj.T @ aoutT
                    with tc.tile_pool(name="prs", bufs=1) as prs, \
                         tc.tile_pool(name="prps", bufs=1, space="PSUM") as pps:
                        x2ps = [pps.tile([128, 512], F32, name=f"x2p{m}")
                                for m in range(CK)]
                        for k in range(CK):
                            at = prs.tile([128, 512], F32R, name="at", bufs=3)
                            nc.sync.dma_start(at[:],
                                              oax[128 * k:128 * (k + 1), :]
                                              .bitcast(F32R))
                            wp = prs.tile([128, C], F32R, name="wp", bufs=3)
                            nc.sync.dma_start(wp[:],
                                              wproj[128 * k:128 * (k + 1), :]
                                              .bitcast(F32R))
                            for m in range(CK):
                                nc.tensor.matmul(
                                    x2ps[m][:], wp[:, 128 * m:128 * (m + 1)],
                                    at[:], start=(k == 0), stop=(k == CK - 1))
                        for m in range(CK):
                            nc.vector.tensor_add(
                                x2T_t[:, 512 * m:512 * (m + 1)], x2ps[m][:],
                                xT_t[:, 512 * m:512 * (m + 1)].bitcast(F32))

                if DEBUG:
                    nc.sync.dma_start(dbg["d_x2T"][:], x2T_t[:].bitcast(F32))

                # ---------------- MLP scope ----------------
                with tc.tile_pool(name="mlp", bufs=1) as mp:
                    h2T_t = mp.tile([128, T], F32R, name="h2T_t")
                    _layer_norm(nc, tc, mp, x2T_t, ln2w_t, h2T_t, eps_t,
                                ones_t, "2")
                    if DEBUG:
                        nc.sync.dma_start(dbg["d_h2T"][:], h2T_t[:].bitcast(F32))
                    gel = []
                    wfc_r = wfc.rearrange("(k p) h -> p k h", p=128)
                    with tc.tile_pool(name="fcs", bufs=1) as fs, \
                         tc.tile_pool(name="fcps", bufs=1, space="PSUM") as fps:
                        for m in range(FC // 128):
                            wg = fs.tile([128, CK, 128], F32R, name="wfcg", bufs=6)
                            nc.sync.dma_start(
                                wg[:], wfc_r[:, :, 128 * m:128 * (m + 1)]
                                .bitcast(F32R))
                            pf = fps.tile([128, 512], F32, name="fcp", bufs=2)
                            for k in range(CK):
                                nc.tensor.matmul(
                                    pf[:], wg[:, k, :],
                                    h2T_t[:, 512 * k:512 * (k + 1)],
                                    start=(k == 0), stop=(k == CK - 1))
                            g = mp.tile([128, 512], F32R, name=f"gel{m}")
                            nc.scalar.activation(g[:], pf[:], AF.Gelu)
                            gel.append(g)
                    # second matmul in two half-passes (PSUM budget): each
                    # half loads its own 512 columns of wmlp
                    with tc.tile_pool(name="m2s", bufs=1) as m2s, \
                         tc.tile_pool(name="m2ps", bufs=1, space="PSUM") as m2ps:
                        for half in range(2):
                            x3ps = [m2ps.tile([128, 512], F32,
                                              name=f"x3p{half}_{i}")
                                    for i in range(4)]
                            for h in range(FC // 128):
                                wm = m2s.tile([128, 512], F32R, name="wm",
                                              bufs=8)
                                nc.sync.dma_start(
                                    wm[:],
                                    wmlp[128 * h:128 * (h + 1),
                                         512 * half:512 * (half + 1)]
                                    .bitcast(F32R))
                                for i in range(4):
                                    m = 4 * half + i
                                    nc.tensor.matmul(
                                        x3ps[i][:],
                                        wm[:, 128 * i:128 * (i + 1)],
                                        gel[h][:], start=(h == 0),
                                        stop=(h == FC // 128 - 1))
                            for i in range(4):
                                m = 4 * half + i
                                nc.vector.tensor_add(
                                    outT_sb[:, 512 * m:512 * (m + 1)],
                                    x3ps[i][:],
                                    x2T_t[:, 512 * m:512 * (m + 1)]
                                    .bitcast(F32))
                for m in range(CK):
                    nc.sync.dma_start(outT[128 * m:128 * (m + 1), :],
                                      outT_sb[:, 512 * m:512 * (m + 1)])

    nc.compile()
    return nc


def _host_inputs(x, w_qkv, w_attn_proj, w_fc, w_mlp_proj, ln1_w, ln2_w):
    """Build the 8 per-core input maps."""
    x2 = np.ascontiguousarray(np.asarray(x, np.float32).reshape(T, C))
    w_qkv = np.asarray(w_qkv, np.float32)
    masks = np.zeros((128, 4 * 512), np.float32)
    kk = np.arange(128)[:, None]
    qq = np.arange(512)[None, :]
    for j in range(4):
        masks[:, 512 * j:512 * (j + 1)] = (qq >= kk + 128 * j)
    masks = masks.astype(ml_dtypes.bfloat16)
    ident = np.eye(128, dtype=np.float32)
    onesw = np.ones((128, 128), np.float32)
    ln1 = np.ascontiguousarray(np.asarray(ln1_w, np.float32).reshape(CK, 128).T)
    ln2 = np.ascontiguousarray(np.asarray(ln2_w, np.float32).reshape(CK, 128).T)
    common = {
        "wproj": np.ascontiguousarray(np.asarray(w_attn_proj, np.float32)),
        "wfc": np.ascontiguousarray(np.asarray(w_fc, np.float32)),
        "wmlp": np.ascontiguousarray(np.asarray(w_mlp_proj, np.float32)),
        "ln1w": ln1, "ln2w": ln2, "masks": masks, "ident": ident,
        "onesw": onesw, "zeros": np.zeros((64, T), np.float32),
    }
    in_maps = []
    for c in range(NCORES):
        xTc = np.ascontiguousarray(x2[TC * c:TC * (c + 1), :].T)
        wq = np.ascontiguousarray(np.concatenate(
            [w_qkv[:, C * s + 128 * c:C * s + 128 * (c + 1)] for s in range(3)],
            axis=1))
        in_maps.append({"xT": xTc, "wqkv": wq, **common})
    return in_maps


def _run(in_maps, **kw):
    key = ("nc", DEBUG)
    if key not in _CACHE:
        _CACHE[key] = _build()
    return bass_utils.run_bass_kernel_spmd(
        _CACHE[key], in_maps, core_ids=list(range(NCORES)), **kw)


def kernel(x, w_qkv, w_attn_proj, w_fc, w_mlp_proj, ln1_w, ln2_w):
    in_maps = _host_inputs(x, w_qkv, w_attn_proj, w_fc, w_mlp_proj,
                           ln1_w, ln2_w)
    res = _run(in_maps)
    out = np.empty((1, T, C), np.float32)
    for c in range(NCORES):
        out[0, TC * c:TC * (c + 1), :] = res.results[c]["outT"].T
    return out



# revision 6
# speedup vs baseline: 1.4629x; 1.4629x over previous
"""Trainium2 Bass kernel for a pre-norm transformer block (B=1, T=4096, C=1024, H=16).

Sharding (8 cores): head-tensor-parallel attention (2 heads/core) with
sequence-parallel LayerNorms and sequence-local (data-parallel) MLP.
Activations are kept transposed on-chip ([C, T] with C on partitions)
so every matmul contracts over the partition axis with K=128 chunks.

Per core c (owns T-columns [512c, 512c+512) and heads 2c, 2c+1):
  1. LN1 on own xT columns -> h1T_c [1024, 512] (bf16)
  2. AllGather h1T (bf16) -> h1all [8192, 512]
  3. qkvT = wqkv_c.T @ h1T_full for the 2 local heads  (full T)
  4. causal flash attention, no max subtraction (scores are O(1)):
     scores for 2 k-tiles land in a double-buffered 2-bank PSUM tile,
     one batched exp per pair (ACT pipelines against PE), PV with
     V-stationary [128,65] (65th col of ones accumulates the softmax
     denominator l) -> unnormalized o plus l per query
  5. AllToAll (bf16) ships unnormalized o + l; receiver computes
     1/l for all 16 heads at once (ACT ln/exp), broadcasts over the
     64 head dims with a tiny matmul, scales, then
     proj: x2T = xT + wproj.T @ aoutT
  6. LN2 -> h2T (bf16); MLP with bf16 weights; out = x2T + mlpT

All matmul operands are bf16 (f32r moving streams at half rate).
"""
import numpy as np
import ml_dtypes

import concourse.bass as bass
import concourse.bacc as bacc
import concourse.tile as tile
import concourse.mybir as mybir
from concourse import bass_utils

F32 = mybir.dt.float32
F32R = mybir.dt.float32r
BF16 = mybir.dt.bfloat16
AF = mybir.ActivationFunctionType
OP = mybir.AluOpType

NCORES = 8
C = 1024
T = 4096
TC = T // NCORES          # 512 own T columns
CK = C // 128             # 8 C chunks
HS = 64
FC = 4096                 # MLP hidden
EPS = 1e-5

_CACHE = {}
DEBUG = False


def _layer_norm(nc, tc, x_t, w_ap, out_t, eps_t, ones_t, ones_b, tag):
    """x_t: [128, CK, 512] F32R tile (C-chunk k at [:, k, :]), w_ap [128,CK],
    out_t [128, CK, 512] BF16. LN over the C axis (partitions x chunks)."""
    with tc.tile_pool(name=f"ln{tag}", bufs=1) as lnp, \
         tc.tile_pool(name=f"lnps{tag}", bufs=1, space="PSUM") as lps:
        mean_ps = lps.tile([128, 512], F32, name=f"mean{tag}")
        sq_ps = lps.tile([128, 512], F32, name=f"sqs{tag}")
        sq_tiles = []
        for k in range(CK):
            sq = lnp.tile([128, 512], BF16, name=f"sq{tag}", bufs=2)
            xk = x_t[:, k, :].bitcast(F32)
            nc.vector.tensor_mul(sq[:], xk, xk)
            sq_tiles.append(sq)
        for k in range(CK):
            nc.tensor.matmul(mean_ps[:], ones_t[:], x_t[:, k, :],
                             start=(k == 0), stop=(k == CK - 1))
        for k in range(CK):
            nc.tensor.matmul(sq_ps[:], ones_b[:], sq_tiles[k][:],
                             start=(k == 0), stop=(k == CK - 1))
        # mu and rstd, broadcast along partitions already (ones-matmul)
        mu = lnp.tile([128, 512], F32, name=f"mu{tag}")
        nc.vector.tensor_scalar_mul(mu[:], mean_ps[:], 1.0 / C)
        musq = lnp.tile([128, 512], F32, name=f"musq{tag}")
        nc.vector.tensor_mul(musq[:], mu[:], mu[:])
        var = lnp.tile([128, 512], F32, name=f"var{tag}")
        nc.vector.scalar_tensor_tensor(var[:], sq_ps[:], 1.0 / C, musq[:],
                                       OP.mult, OP.subtract)
        lnv = lnp.tile([128, 512], F32, name=f"lnv{tag}")
        nc.scalar.activation(lnv[:], var[:], AF.Ln, bias=eps_t[:])
        rstd = lnp.tile([128, 512], F32, name=f"rstd{tag}")
        nc.scalar.activation(rstd[:], lnv[:], AF.Exp, scale=-0.5)
        for k in range(CK):
            d = lnp.tile([128, 512], F32, name=f"d{tag}", bufs=2)
            nc.vector.tensor_sub(d[:], x_t[:, k, :].bitcast(F32), mu[:])
            nc.vector.scalar_tensor_tensor(
                out_t[:, k, :], d[:], w_ap[:, k:k + 1],
                rstd[:], OP.mult, OP.mult)


def _build():
    nc = bacc.Bacc("TRN2", target_bir_lowering=False, debug=False,
                   enable_asserts=False, num_devices=NCORES)

    xT = nc.dram_tensor("xT", [C, TC], F32, kind="ExternalInput").ap()
    wqkv = nc.dram_tensor("wqkv", [128, CK, 3 * 128], BF16,
                          kind="ExternalInput").ap()
    wproj = nc.dram_tensor("wproj", [128, CK, C], BF16,
                           kind="ExternalInput").ap()
    wfc = nc.dram_tensor("wfc", [128, FC // 128, CK, 128], BF16,
                         kind="ExternalInput").ap()
    wmlp = nc.dram_tensor("wmlp", [128, FC // 128, C], BF16,
                          kind="ExternalInput").ap()
    ln1w = nc.dram_tensor("ln1w", [128, CK], F32, kind="ExternalInput").ap()
    ln2w = nc.dram_tensor("ln2w", [128, CK], F32, kind="ExternalInput").ap()
    masks = nc.dram_tensor("masks", [128, 4 * 512], BF16,
                           kind="ExternalInput").ap()
    ident = nc.dram_tensor("ident", [128, 128], BF16, kind="ExternalInput").ap()
    onesw = nc.dram_tensor("onesw", [128, 128], F32, kind="ExternalInput").ap()
    sel16 = nc.dram_tensor("sel16", [16, CK * 128], BF16,
                           kind="ExternalInput").ap()
    zeros = nc.dram_tensor("zeros", [64, T], BF16, kind="ExternalInput").ap()
    outT = nc.dram_tensor("outT", [C, TC], F32, kind="ExternalOutput").ap()
    dbg = {}
    if DEBUG:
        for nm in ("d_h1T", "d_qp0", "d_qp1", "d_kT", "d_vT"):
            dbg[nm] = nc.dram_tensor(nm, [128, T], BF16,
                                     kind="ExternalOutput").ap()
        dbg["d_oTe0"] = nc.dram_tensor("d_oTe0", [65, T], BF16,
                                       kind="ExternalOutput").ap()
        dbg["d_oTe1"] = nc.dram_tensor("d_oTe1", [65, T], BF16,
                                       kind="ExternalOutput").ap()
        dbg["d_lall"] = nc.dram_tensor("d_lall", [16, TC], BF16,
                                       kind="ExternalOutput").ap()
        dbg["d_rlall"] = nc.dram_tensor("d_rlall", [16, TC], BF16,
                                        kind="ExternalOutput").ap()
        dbg["d_at"] = nc.dram_tensor("d_at", [128, CK * TC], BF16,
                                     kind="ExternalOutput").ap()
        dbg["d_x2T"] = nc.dram_tensor("d_x2T", [128, T], F32,
                                      kind="ExternalOutput").ap()
        dbg["d_h2T"] = nc.dram_tensor("d_h2T", [128, T], BF16,
                                      kind="ExternalOutput").ap()

    rg = [list(range(NCORES))]

    with tile.TileContext(nc) as tc:
        with tc.tile_pool(name="dram", bufs=1, space="DRAM") as dramp:
            h1b = dramp.tile([C, TC], BF16, name="h1b")
            h1all = dramp.tile([NCORES * C, TC], BF16, name="h1all",
                               addr_space="Shared")
            # AllToAll payload: per 130-row block: 128 unnormalized o rows
            # (head a dims 0:64, head b dims 64:128), then 2 l rows.
            ob = dramp.tile([NCORES * 130, TC], BF16, name="ob")
            oax = dramp.tile([NCORES * 130, TC], BF16, name="oax")

            with tc.tile_pool(name="glob", bufs=1) as gp:
                ident_t = gp.tile([128, 128], BF16, name="ident_t")
                nc.sync.dma_start(ident_t[:], ident[:])
                ones_t = gp.tile([128, 128], F32R, name="ones_t")
                nc.sync.dma_start(ones_t[:], onesw[:].bitcast(F32R))
                ones_b = gp.tile([128, 128], BF16, name="ones_b")
                nc.vector.memset(ones_b[:], 1.0)
                sel16_t = gp.tile([16, CK * 128], BF16, name="sel16_t")
                nc.sync.dma_start(sel16_t[:], sel16[:])
                ln1w_t = gp.tile([128, CK], F32, name="ln1w_t")
                nc.sync.dma_start(ln1w_t[:], ln1w[:])
                ln2w_t = gp.tile([128, CK], F32, name="ln2w_t")
                nc.sync.dma_start(ln2w_t[:], ln2w[:])
                masks_t = gp.tile([128, 4 * 512], BF16, name="masks_t")
                nc.sync.dma_start(masks_t[:], masks[:])
                eps_t = gp.tile([128, 1], F32, name="eps_t")
                nc.vector.memset(eps_t[:], EPS)
                # weights used before the MLP phase: preload in one DMA each
                wq_t = gp.tile([128, CK, 3 * 128], BF16, name="wq_t")
                nc.sync.dma_start(wq_t[:], wqkv[:])
                wp_t = gp.tile([128, CK, C], BF16, name="wp_t")
                nc.sync.dma_start(wp_t[:], wproj[:])
                xT_t = gp.tile([128, CK, 512], F32R, name="xT_t")
                nc.sync.dma_start(
                    xT_t[:], xT.rearrange("(k p) c -> p k c", p=128)
                    .bitcast(F32R))
                x2T_t = gp.tile([128, CK, 512], F32R, name="x2T_t")
                outT_sb = gp.tile([128, CK, 512], F32, name="outT_sb")

                # ---------------- attention scope ----------------
                with tc.tile_pool(name="attn", bufs=1) as ap:
                    h1T_t = ap.tile([128, CK, 512], BF16, name="h1T_t")
                    _layer_norm(nc, tc, xT_t, ln1w_t, h1T_t, eps_t,
                                ones_t, ones_b, "1")
                    nc.sync.dma_start(
                        h1b.rearrange("(k p) c -> p k c", p=128), h1T_t[:])
                    nc.gpsimd.collective_compute(
                        "AllGather", OP.bypass, replica_groups=rg,
                        ins=[h1b.opt()], outs=[h1all.opt()])

                    # persistent attention tensors
                    qp0 = ap.tile([128, T], BF16, name="qp0")
                    qp1 = ap.tile([128, T], BF16, name="qp1")
                    kT_t = ap.tile([128, T], BF16, name="kT_t")
                    vT_t = ap.tile([128, T], BF16, name="vT_t")
                    # unnormalized head outputs + l (row 64), one per head
                    oTe0 = ap.tile([65, T], BF16, name="oTe0")
                    oTe1 = ap.tile([65, T], BF16, name="oTe1")
                    nc.sync.dma_start(qp0[64:128, :], zeros[:])
                    nc.sync.dma_start(qp1[0:64, :], zeros[:])

                    # qkvT: for each T-block j, accumulate over C chunks
                    h1all_r = h1all.rearrange("(j k p) c -> j p k c",
                                              p=128, k=CK)
                    with tc.tile_pool(name="qkvs", bufs=1) as qs, \
                         tc.tile_pool(name="qkvps", bufs=1, space="PSUM") as qps:
                        for j in range(NCORES):
                            hr = qs.tile([128, CK, 512], BF16, name="h1rt",
                                         bufs=3)
                            nc.sync.dma_start(hr[:], h1all_r[j])
                            blk = slice(512 * j, 512 * (j + 1))
                            for m in range(3):
                                pm = qps.tile([128, 512], F32, name="qkvp",
                                              bufs=3)
                                for k in range(CK):
                                    nc.tensor.matmul(
                                        pm[:],
                                        wq_t[:, k, 128 * m:128 * (m + 1)],
                                        hr[:, k, :], start=(k == 0),
                                        stop=(k == CK - 1))
                                if m == 0:
                                    nc.scalar.activation(qp0[0:64, blk],
                                                         pm[0:64, :], AF.Copy)
                                    nc.scalar.activation(qp1[64:128, blk],
                                                         pm[64:128, :],
                                                         AF.Copy)
                                elif m == 1:
                                    nc.scalar.activation(kT_t[:, blk], pm[:],
                                                         AF.Copy)
                                else:
                                    nc.vector.tensor_copy(vT_t[:, blk], pm[:])

                    # v_ext: transpose vT into per-head [kpos,64]+ones tiles
                    ve = [[], []]
                    with tc.tile_pool(name="veps", bufs=1, space="PSUM") as vps:
                        for t in range(T // 128):
                            tp = vps.tile([128, 128], BF16, name="vtp", bufs=2)
                            nc.tensor.transpose(tp[:],
                                                vT_t[:, 128 * t:128 * (t + 1)],
                                                ident_t[:])
                            for h in range(2):
                                vx = ap.tile([128, 65], BF16, name=f"ve{h}_{t}")
                                nc.vector.tensor_copy(
                                    vx[:, 0:64], tp[:, 64 * h:64 * (h + 1)])
                                nc.vector.memset(vx[:, 64:65], 1.0)
                                ve[h].append(vx)

                    # flash attention (no max subtraction), 2 heads.
                    # scores for 2 k-tiles land in a double-buffered 2-bank
                    # PSUM tile; one batched exp per pair pipelines ACT
                    # against PE; then V-stationary PV accumulation.
                    qp = [qp0, qp1]
                    oTe = [oTe0, oTe1]
                    with tc.tile_pool(name="atw", bufs=1) as aw, \
                         tc.tile_pool(name="atps", bufs=1, space="PSUM") as aps, \
                         tc.tile_pool(name="atpo", bufs=1, space="PSUM") as apo:
                        for qi in range(NCORES):
                            npr = 2 * (qi + 1)
                            for h in range(2):
                                qsl = qp[h][:, 512 * qi:512 * (qi + 1)]
                                ops = apo.tile([65, 512], F32, name="ops",
                                               bufs=2)
                                for p in range(npr):
                                    sp = aps.tile([128, 2, 512], F32,
                                                  name="sp", bufs=2)
                                    for u in range(2):
                                        kb = 2 * p + u
                                        nc.tensor.matmul(
                                            sp[:, u, :],
                                            kT_t[:, 128 * kb:128 * (kb + 1)],
                                            qsl, start=True, stop=True)
                                    est = aw.tile([128, 2, 512], BF16,
                                                  name="est", bufs=4)
                                    nc.scalar.activation(est[:], sp[:],
                                                         AF.Exp, scale=0.125)
                                    if p >= npr - 2:  # diagonal: causal mask
                                        for u in range(2):
                                            jm = 2 * p + u - 4 * qi
                                            nc.vector.tensor_mul(
                                                est[:, u, :], est[:, u, :],
                                                masks_t[:, 512 * jm:512 * (jm + 1)])
                                    for u in range(2):
                                        kb = 2 * p + u
                                        nc.tensor.matmul(
                                            ops[:], ve[h][kb][:],
                                            est[:, u, :],
                                            start=(p == 0 and u == 0),
                                            stop=(p == npr - 1 and u == 1))
                                nc.vector.tensor_copy(
                                    oTe[h][:, 512 * qi:512 * (qi + 1)], ops[:])

                    if DEBUG:
                        nc.sync.dma_start(dbg["d_h1T"][:], h1T_t[:])
                        nc.sync.dma_start(dbg["d_qp0"][:], qp0[:])
                        nc.sync.dma_start(dbg["d_qp1"][:], qp1[:])
                        nc.sync.dma_start(dbg["d_kT"][:], kT_t[:])
                        nc.sync.dma_start(dbg["d_vT"][:], vT_t[:])
                        nc.sync.dma_start(dbg["d_oTe0"][:], oTe0[:])
                        nc.sync.dma_start(dbg["d_oTe1"][:], oTe1[:])

                    # exchange head outputs + l rows: AllToAll
                    for j in range(NCORES):
                        blk = slice(512 * j, 512 * (j + 1))
                        base = 130 * j
                        nc.sync.dma_start(ob[base:base + 64, :],
                                          oTe0[0:64, blk])
                        nc.sync.dma_start(ob[base + 64:base + 128, :],
                                          oTe1[0:64, blk])
                        nc.sync.dma_start(ob[base + 128:base + 129, :],
                                          oTe0[64:65, blk])
                        nc.sync.dma_start(ob[base + 129:base + 130, :],
                                          oTe1[64:65, blk])
                    nc.gpsimd.collective_compute(
                        "AllToAll", OP.bypass, replica_groups=rg,
                        ins=[ob.opt()], outs=[oax.opt()])

                    # proj: x2T = xT + wproj.T @ (aoutT * 1/l)
                    with tc.tile_pool(name="prs", bufs=1) as prs, \
                         tc.tile_pool(name="prps", bufs=1, space="PSUM") as pps:
                        # all 16 heads' l for own columns -> one 1/l pass
                        l_all = prs.tile([16, 512], BF16, name="l_all")
                        for k in range(CK):
                            nc.sync.dma_start(
                                l_all[2 * k:2 * k + 2, :],
                                oax[130 * k + 128:130 * k + 130, :])
                        lln = prs.tile([16, 512], F32, name="lln")
                        nc.scalar.activation(lln[:], l_all[:], AF.Ln)
                        rl_all = prs.tile([16, 512], BF16, name="rl_all")
                        nc.scalar.activation(rl_all[:], lln[:], AF.Exp,
                                             scale=-1.0)
                        at_tiles = []
                        for k in range(CK):
                            au = prs.tile([128, 512], BF16, name="au", bufs=3)
                            nc.sync.dma_start(au[:],
                                              oax[130 * k:130 * k + 128, :])
                            rlb = pps.tile([128, 512], F32, name="rlb", bufs=2)
                            nc.tensor.matmul(
                                rlb[:], sel16_t[:, 128 * k:128 * (k + 1)],
                                rl_all[:], start=True, stop=True)
                            at = prs.tile([128, 512], BF16, name=f"at{k}")
                            nc.vector.tensor_mul(at[:], au[:], rlb[:])
                            at_tiles.append(at)
                        if DEBUG:
                            nc.sync.dma_start(dbg["d_lall"][:], l_all[:])
                            nc.sync.dma_start(dbg["d_rlall"][:], rl_all[:])
                            for k in range(CK):
                                nc.sync.dma_start(
                                    dbg["d_at"][:, 512 * k:512 * (k + 1)],
                                    at_tiles[k][:])
                        for m in range(CK):
                            x2ps = pps.tile([128, 512], F32, name="x2p",
                                            bufs=2)
                            for k in range(CK):
                                nc.tensor.matmul(
                                    x2ps[:], wp_t[:, k, 128 * m:128 * (m + 1)],
                                    at_tiles[k][:], start=(k == 0),
                                    stop=(k == CK - 1))
                            nc.vector.tensor_add(
                                x2T_t[:, m, :], x2ps[:],
                                xT_t[:, m, :].bitcast(F32))

                if DEBUG:
                    nc.sync.dma_start(dbg["d_x2T"][:], x2T_t[:].bitcast(F32))

                # ---------------- MLP scope ----------------
                with tc.tile_pool(name="mlp", bufs=1) as mp:
                    h2T_t = mp.tile([128, CK, 512], BF16, name="h2T_t")
                    _layer_norm(nc, tc, x2T_t, ln2w_t, h2T_t, eps_t,
                                ones_t, ones_b, "2")
                    if DEBUG:
                        nc.sync.dma_start(dbg["d_h2T"][:], h2T_t[:])
                    gel = []
                    with tc.tile_pool(name="fcs", bufs=1) as fs, \
                         tc.tile_pool(name="fcps", bufs=1, space="PSUM") as fps:
                        for g in range(FC // 512):  # 8 groups of 4 m-blocks
                            wg = fs.tile([128, 4, CK, 128], BF16, name="wfcg",
                                         bufs=2)
                            nc.sync.dma_start(wg[:], wfc[:, 4 * g:4 * g + 4])
                            pf = fps.tile([128, 4, 512], F32, name="fcp",
                                          bufs=2)
                            for mm in range(4):
                                for k in range(CK):
                                    nc.tensor.matmul(
                                        pf[:, mm, :], wg[:, mm, k, :],
                                        h2T_t[:, k, :],
                                        start=(k == 0), stop=(k == CK - 1))
                            gl = mp.tile([128, 4, 512], BF16, name=f"gel{g}")
                            nc.scalar.activation(gl[:], pf[:], AF.Gelu)
                            gel.append(gl)
                    # second matmul: single pass, 8 psum accumulators
                    with tc.tile_pool(name="m2s", bufs=1) as m2s, \
                         tc.tile_pool(name="m2ps", bufs=1, space="PSUM") as m2ps:
                        x3ps = [m2ps.tile([128, 512], F32, name=f"x3p{i}")
                                for i in range(CK)]
                        for f4 in range(FC // 512):
                            wm = m2s.tile([128, 4, C], BF16, name="wm",
                                          bufs=2)
                            nc.sync.dma_start(wm[:],
                                              wmlp[:, 4 * f4:4 * f4 + 4, :])
                            for ff in range(4):
                                f = 4 * f4 + ff
                                for i in range(CK):
                                    nc.tensor.matmul(
                                        x3ps[i][:],
                                        wm[:, ff, 128 * i:128 * (i + 1)],
                                        gel[f // 4][:, f % 4, :],
                                        start=(f == 0),
                                        stop=(f == FC // 128 - 1))
                        for i in range(CK):
                            nc.vector.tensor_add(
                                outT_sb[:, i, :], x3ps[i][:],
                                x2T_t[:, i, :].bitcast(F32))
                nc.sync.dma_start(
                    outT.rearrange("(k p) c -> p k c", p=128), outT_sb[:])

    nc.compile()
    return nc


def _host_inputs(x, w_qkv, w_attn_proj, w_fc, w_mlp_proj, ln1_w, ln2_w):
    """Build the 8 per-core input maps."""
    bf = ml_dtypes.bfloat16
    x2 = np.ascontiguousarray(np.asarray(x, np.float32).reshape(T, C))
    w_qkv = np.asarray(w_qkv, np.float32)
    masks = np.zeros((128, 4 * 512), np.float32)
    kk = np.arange(128)[:, None]
    qq = np.arange(512)[None, :]
    for j in range(4):
        masks[:, 512 * j:512 * (j + 1)] = (qq >= kk + 128 * j)
    masks = masks.astype(bf)
    ident = np.eye(128, dtype=np.float32).astype(bf)
    onesw = np.ones((128, 128), np.float32)
    # sel16[:, 128k + d] = 1 where row r == 2k + d//64 (head of dim d in
    # aout chunk k); broadcasts rl_all rows onto the head-dim rows.
    sel16 = np.zeros((16, CK * 128), np.float32)
    for k in range(CK):
        sel16[2 * k, 128 * k:128 * k + 64] = 1.0
        sel16[2 * k + 1, 128 * k + 64:128 * (k + 1)] = 1.0
    sel16 = sel16.astype(bf)
    ln1 = np.ascontiguousarray(np.asarray(ln1_w, np.float32).reshape(CK, 128).T)
    ln2 = np.ascontiguousarray(np.asarray(ln2_w, np.float32).reshape(CK, 128).T)
    wproj = np.asarray(w_attn_proj, np.float32).reshape(CK, 128, C) \
        .transpose(1, 0, 2).astype(bf)
    wfc = np.asarray(w_fc, np.float32).reshape(CK, 128, FC // 128, 128) \
        .transpose(1, 2, 0, 3).astype(bf)
    wmlp = np.asarray(w_mlp_proj, np.float32).reshape(FC // 128, 128, C) \
        .transpose(1, 0, 2).astype(bf)
    common = {
        "wproj": np.ascontiguousarray(wproj),
        "wfc": np.ascontiguousarray(wfc),
        "wmlp": np.ascontiguousarray(wmlp),
        "ln1w": ln1, "ln2w": ln2, "masks": masks, "ident": ident,
        "onesw": onesw, "sel16": sel16,
        "zeros": np.zeros((64, T), bf),
    }
    in_maps = []
    for c in range(NCORES):
        xTc = np.ascontiguousarray(x2[TC * c:TC * (c + 1), :].T)
        wq = np.concatenate(
            [w_qkv[:, C * s + 128 * c:C * s + 128 * (c + 1)] for s in range(3)],
            axis=1)  # [C, 384]
        wq = np.ascontiguousarray(
            wq.reshape(CK, 128, 3 * 128).transpose(1, 0, 2).astype(bf))
        in_maps.append({"xT": xTc, "wqkv": wq, **common})
    return in_maps


def _run(in_maps, **kw):
    key = ("nc", DEBUG)
    if key not in _CACHE:
        _CACHE[key] = _build()
    return bass_utils.run_bass_kernel_spmd(
        _CACHE[key], in_maps, core_ids=list(range(NCORES)), **kw)


def kernel(x, w_qkv, w_attn_proj, w_fc, w_mlp_proj, ln1_w, ln2_w):
    in_maps = _host_inputs(x, w_qkv, w_attn_proj, w_fc, w_mlp_proj,
                           ln1_w, ln2_w)
    res = _run(in_maps)
    out = np.empty((1, T, C), np.float32)
    for c in range(NCORES):
        out[0, TC * c:TC * (c + 1), :] = res.results[c]["outT"].T
    return out


# revision 10
# speedup vs baseline: 1.5619x; 1.0677x over previous
"""Trainium2 Bass kernel for a pre-norm transformer block (B=1, T=4096, C=1024, H=16).

Sharding (8 cores): head-tensor-parallel attention (2 heads/core) with
sequence-parallel LayerNorm statistics and sequence-local MLP.
Activations are kept transposed on-chip ([C, T] with C on partitions)
so every matmul contracts over the partition axis with K=128 chunks.

Per core c (owns T-columns [512c, 512c+512) and heads 2c, 2c+1):
  1. LN1 statistics on own xT columns -> mu, rstd [1, 512]
  2. Tiny AllGather of (rstd, mu*rstd) [1,1024] bf16 -> [8,1024].
     LN is affine in x, so  qkv = rstd*(x @ W') - (mu*rstd)*colsum(W')
     with W' = diag(ln1_w) @ w_qkv folded host-side: QKV for the full
     sequence is computed locally from a prefetched bf16 copy of x —
     no big h1 AllGather.
  3. causal flash attention, no max subtraction (scores are O(1)):
     scores for 2 k-tiles land in a triple-buffered 2-bank PSUM tile,
     one batched exp per pair (ACT pipelines against PE), PV with
     V-stationary [128,65] (65th col of ones accumulates the softmax
     denominator l) -> unnormalized o plus l per query
  4. AllToAll (bf16) ships unnormalized o + l; receiver computes
     1/l for all 16 heads at once (ACT ln/exp), broadcasts over the
     64 head dims with a tiny matmul, scales, then
     proj: x2T = xT + wproj.T @ aoutT
  5. LN2 -> h2T (bf16); MLP with bf16 weights; out = x2T + mlpT

All matmul operands are bf16 (f32r moving streams at half rate).
"""
import numpy as np
import ml_dtypes

import concourse.bass as bass
import concourse.bacc as bacc
import concourse.tile as tile
import concourse.mybir as mybir
from concourse import bass_utils

F32 = mybir.dt.float32
F32R = mybir.dt.float32r
BF16 = mybir.dt.bfloat16
AF = mybir.ActivationFunctionType
OP = mybir.AluOpType

NCORES = 8
C = 1024
T = 4096
TC = T // NCORES          # 512 own T columns
CK = C // 128             # 8 C chunks
HS = 64
FC = 4096                 # MLP hidden
EPS = 1e-5

_CACHE = {}
DEBUG = False


def _ln_stats(nc, tc, x_t, eps_t, ones_t, ones_b, mr, tag):
    """x_t: [128, CK, 512] F32R. Writes mr [1, 1024] BF16:
    cols 0:512 = rstd, 512:1024 = mu*rstd for the 512 own tokens."""
    with tc.tile_pool(name=f"ln{tag}", bufs=1) as lnp, \
         tc.tile_pool(name=f"lnps{tag}", bufs=1, space="PSUM") as lps:
        mean_ps = lps.tile([128, 512], F32, name=f"mean{tag}")
        sq_ps = lps.tile([128, 512], F32, name=f"sqs{tag}")
        sq_tiles = []
        for k in range(CK):
            sq = lnp.tile([128, 512], BF16, name=f"sq{tag}", bufs=2)
            xk = x_t[:, k, :].bitcast(F32)
            nc.vector.tensor_mul(sq[:], xk, xk)
            sq_tiles.append(sq)
        for k in range(CK):
            nc.tensor.matmul(mean_ps[:], ones_t[:], x_t[:, k, :],
                             start=(k == 0), stop=(k == CK - 1))
        for k in range(CK):
            nc.tensor.matmul(sq_ps[:], ones_b[:], sq_tiles[k][:],
                             start=(k == 0), stop=(k == CK - 1))
        mu = lnp.tile([1, 512], F32, name=f"mu{tag}")
        nc.vector.tensor_scalar_mul(mu[:], mean_ps[0:1, :], 1.0 / C)
        musq = lnp.tile([1, 512], F32, name=f"musq{tag}")
        nc.vector.tensor_mul(musq[:], mu[:], mu[:])
        var = lnp.tile([1, 512], F32, name=f"var{tag}")
        nc.vector.scalar_tensor_tensor(var[:], sq_ps[0:1, :], 1.0 / C,
                                       musq[:], OP.mult, OP.subtract)
        lnv = lnp.tile([1, 512], F32, name=f"lnv{tag}")
        nc.scalar.activation(lnv[:], var[:], AF.Ln, bias=eps_t[0:1, :])
        rstd = lnp.tile([1, 512], F32, name=f"rstd{tag}")
        nc.scalar.activation(rstd[:], lnv[:], AF.Exp, scale=-0.5)
        nc.vector.tensor_copy(mr[:, 0:512], rstd[:])
        nc.vector.tensor_mul(mr[:, 512:1024], mu[:], rstd[:])


def _layer_norm(nc, tc, x_t, w_ap, out_t, eps_t, ones_t, ones_b, tag):
    """Full LN producing out_t [128, CK, 512] BF16 (used for LN2)."""
    with tc.tile_pool(name=f"ln{tag}", bufs=1) as lnp, \
         tc.tile_pool(name=f"lnps{tag}", bufs=1, space="PSUM") as lps:
        mean_ps = lps.tile([128, 512], F32, name=f"mean{tag}")
        sq_ps = lps.tile([128, 512], F32, name=f"sqs{tag}")
        sq_tiles = []
        for k in range(CK):
            sq = lnp.tile([128, 512], BF16, name=f"sq{tag}", bufs=2)
            xk = x_t[:, k, :].bitcast(F32)
            nc.vector.tensor_mul(sq[:], xk, xk)
            sq_tiles.append(sq)
        for k in range(CK):
            nc.tensor.matmul(mean_ps[:], ones_t[:], x_t[:, k, :],
                             start=(k == 0), stop=(k == CK - 1))
        for k in range(CK):
            nc.tensor.matmul(sq_ps[:], ones_b[:], sq_tiles[k][:],
                             start=(k == 0), stop=(k == CK - 1))
        mu = lnp.tile([128, 512], F32, name=f"mu{tag}")
        nc.vector.tensor_scalar_mul(mu[:], mean_ps[:], 1.0 / C)
        musq = lnp.tile([128, 512], F32, name=f"musq{tag}")
        nc.vector.tensor_mul(musq[:], mu[:], mu[:])
        var = lnp.tile([128, 512], F32, name=f"var{tag}")
        nc.vector.scalar_tensor_tensor(var[:], sq_ps[:], 1.0 / C, musq[:],
                                       OP.mult, OP.subtract)
        lnv = lnp.tile([128, 512], F32, name=f"lnv{tag}")
        nc.scalar.activation(lnv[:], var[:], AF.Ln, bias=eps_t[:])
        rstd = lnp.tile([128, 512], F32, name=f"rstd{tag}")
        nc.scalar.activation(rstd[:], lnv[:], AF.Exp, scale=-0.5)
        for k in range(CK):
            d = lnp.tile([128, 512], F32, name=f"d{tag}", bufs=2)
            nc.vector.tensor_sub(d[:], x_t[:, k, :].bitcast(F32), mu[:])
            nc.vector.scalar_tensor_tensor(
                out_t[:, k, :], d[:], w_ap[:, k:k + 1],
                rstd[:], OP.mult, OP.mult)


def _build():
    nc = bacc.Bacc("TRN2", target_bir_lowering=False, debug=False,
                   enable_asserts=False, num_devices=NCORES)

    xT = nc.dram_tensor("xT", [C, TC], F32, kind="ExternalInput").ap()
    xb = nc.dram_tensor("xb", [128, CK, T], BF16, kind="ExternalInput").ap()
    wqkv = nc.dram_tensor("wqkv", [128, CK, 3 * 128], BF16,
                          kind="ExternalInput").ap()
    wqsn = nc.dram_tensor("wqsn", [128, 3], F32, kind="ExternalInput").ap()
    wproj = nc.dram_tensor("wproj", [128, CK, C], BF16,
                           kind="ExternalInput").ap()
    wfc = nc.dram_tensor("wfc", [128, FC // 128, CK, 128], BF16,
                         kind="ExternalInput").ap()
    wmlp = nc.dram_tensor("wmlp", [128, FC // 128, C], BF16,
                          kind="ExternalInput").ap()
    ln2w = nc.dram_tensor("ln2w", [128, CK], F32, kind="ExternalInput").ap()
    masks = nc.dram_tensor("masks", [128, 4 * 512], BF16,
                           kind="ExternalInput").ap()
    ident = nc.dram_tensor("ident", [128, 128], BF16, kind="ExternalInput").ap()
    onesw = nc.dram_tensor("onesw", [128, 128], F32, kind="ExternalInput").ap()
    sel16 = nc.dram_tensor("sel16", [16, CK * 128], BF16,
                           kind="ExternalInput").ap()
    zeros = nc.dram_tensor("zeros", [64, T], BF16, kind="ExternalInput").ap()
    outT = nc.dram_tensor("outT", [C, TC], F32, kind="ExternalOutput").ap()
    dbg = {}
    if DEBUG:
        for nm in ("d_qp0", "d_qp1", "d_kT", "d_vT"):
            dbg[nm] = nc.dram_tensor(nm, [128, T], BF16,
                                     kind="ExternalOutput").ap()
        dbg["d_mrall"] = nc.dram_tensor("d_mrall", [8, 1024], BF16,
                                        kind="ExternalOutput").ap()
        dbg["d_oTe0"] = nc.dram_tensor("d_oTe0", [65, T], BF16,
                                       kind="ExternalOutput").ap()
        dbg["d_oTe1"] = nc.dram_tensor("d_oTe1", [65, T], BF16,
                                       kind="ExternalOutput").ap()
        dbg["d_x2T"] = nc.dram_tensor("d_x2T", [128, T], F32,
                                      kind="ExternalOutput").ap()

    rg = [list(range(NCORES))]

    with tile.TileContext(nc) as tc:
        with tc.tile_pool(name="dram", bufs=1, space="DRAM") as dramp:
            mrb = dramp.tile([1, 1024], BF16, name="mrb")
            mrall_d = dramp.tile([NCORES, 1024], BF16, name="mrall_d",
                                 addr_space="Shared")
            # AllToAll payload: per 130-row block: 128 unnormalized o rows
            # (head a dims 0:64, head b dims 64:128), then 2 l rows.
            ob = dramp.tile([NCORES * 130, TC], BF16, name="ob")
            oax = dramp.tile([NCORES * 130, TC], BF16, name="oax")

            with tc.tile_pool(name="glob", bufs=1) as gp:
                ident_t = gp.tile([128, 128], BF16, name="ident_t")
                nc.sync.dma_start(ident_t[:], ident[:])
                ones_t = gp.tile([128, 128], F32R, name="ones_t")
                nc.sync.dma_start(ones_t[:], onesw[:].bitcast(F32R))
                ones_b = gp.tile([128, 128], BF16, name="ones_b")
                nc.vector.memset(ones_b[:], 1.0)
                sel16_t = gp.tile([16, CK * 128], BF16, name="sel16_t")
                nc.sync.dma_start(sel16_t[:], sel16[:])
                ln2w_t = gp.tile([128, CK], F32, name="ln2w_t")
                nc.sync.dma_start(ln2w_t[:], ln2w[:])
                masks_t = gp.tile([128, 4 * 512], BF16, name="masks_t")
                nc.sync.dma_start(masks_t[:], masks[:])
                eps_t = gp.tile([128, 1], F32, name="eps_t")
                nc.vector.memset(eps_t[:], EPS)
                wq_t = gp.tile([128, CK, 3 * 128], BF16, name="wq_t")
                nc.sync.dma_start(wq_t[:], wqkv[:])
                wqsn_t = gp.tile([128, 3], F32, name="wqsn_t")
                nc.sync.dma_start(wqsn_t[:], wqsn[:])
                xT_t = gp.tile([128, CK, 512], F32R, name="xT_t")
                nc.sync.dma_start(
                    xT_t[:], xT.rearrange("(k p) c -> p k c", p=128)
                    .bitcast(F32R))
                x2T_t = gp.tile([128, CK, 512], F32R, name="x2T_t")

                # ---------------- attention scope ----------------
                with tc.tile_pool(name="attn", bufs=1) as ap:
                    mr = ap.tile([1, 1024], BF16, name="mr")
                    _ln_stats(nc, tc, xT_t, eps_t, ones_t, ones_b, mr, "1")
                    nc.sync.dma_start(mrb[:], mr[:])
                    nc.gpsimd.collective_compute(
                        "AllGather", OP.bypass, replica_groups=rg,
                        ins=[mrb.opt()], outs=[mrall_d.opt()])
                    mr_js = []
                    for j in range(NCORES):
                        mr_j = ap.tile([1, 1024], BF16, name=f"mr{j}")
                        nc.sync.dma_start(mr_j[:], mrall_d[j:j + 1, :])
                        mr_js.append(mr_j)

                    # persistent attention tensors
                    qp0 = ap.tile([128, T], BF16, name="qp0")
                    qp1 = ap.tile([128, T], BF16, name="qp1")
                    kT_t = ap.tile([128, T], BF16, name="kT_t")
                    vT_t = ap.tile([128, T], BF16, name="vT_t")
                    # unnormalized head outputs + l (row 64), one per head
                    oTe0 = ap.tile([65, T], BF16, name="oTe0")
                    oTe1 = ap.tile([65, T], BF16, name="oTe1")
                    nc.sync.dma_start(qp0[64:128, :], zeros[:])
                    nc.sync.dma_start(qp1[0:64, :], zeros[:])

                    # qkvT from raw x: qkv = rstd*(x@W') - (mu*rstd)*colsum(W')
                    qdst = [None, kT_t, vT_t]
                    with tc.tile_pool(name="xbp", bufs=1) as xbp, \
                         tc.tile_pool(name="qkvps", bufs=1, space="PSUM") as qps, \
                         tc.tile_pool(name="qbps", bufs=1, space="PSUM") as qbp:
                        xb_t = xbp.tile([128, CK, T], BF16, name="xb_t")
                        nc.sync.dma_start(xb_t[:], xb[:])
                        # per-block broadcast of (rstd, mu*rstd) to 128 rows
                        rbmb = []
                        for j in range(NCORES):
                            bps = qbp.tile([128, 2, 512], F32, name="bps",
                                           bufs=2)
                            nc.tensor.matmul(bps[:, 0, :], ones_b[0:1, :],
                                             mr_js[j][:, 0:512],
                                             start=True, stop=True)
                            nc.tensor.matmul(bps[:, 1, :], ones_b[0:1, :],
                                             mr_js[j][:, 512:1024],
                                             start=True, stop=True)
                            rb = xbp.tile([128, 1024], BF16, name=f"rb{j}")
                            nc.scalar.activation(rb[:], bps[:], AF.Copy)
                            rbmb.append(rb)
                        for jh in range(2):
                            js = [4 * jh + i for i in range(4)]
                            for m in range(3):
                                pms = []
                                for j in js:
                                    pm = qps.tile([128, 512], F32,
                                                  name="qkvp", bufs=2)
                                    pms.append(pm)
                                for k in range(CK):
                                    for ji, j in enumerate(js):
                                        nc.tensor.matmul(
                                            pms[ji][:],
                                            wq_t[:, k, 128 * m:128 * (m + 1)],
                                            xb_t[:, k, 512 * j:512 * (j + 1)],
                                            start=(k == 0),
                                            stop=(k == CK - 1))
                                for ji, j in enumerate(js):
                                    blk = slice(512 * j, 512 * (j + 1))
                                    t1 = xbp.tile([128, 512], BF16, name="t1",
                                                  bufs=3)
                                    nc.vector.tensor_mul(
                                        t1[:], pms[ji][:], rbmb[j][:, 0:512])
                                    if m == 0:
                                        nc.vector.scalar_tensor_tensor(
                                            qp0[0:64, blk],
                                            rbmb[j][0:64, 512:1024],
                                            wqsn_t[0:64, 0:1], t1[0:64, :],
                                            OP.mult, OP.add)
                                        nc.vector.scalar_tensor_tensor(
                                            qp1[64:128, blk],
                                            rbmb[j][64:128, 512:1024],
                                            wqsn_t[64:128, 0:1],
                                            t1[64:128, :], OP.mult, OP.add)
                                    else:
                                        nc.vector.scalar_tensor_tensor(
                                            qdst[m][:, blk], rbmb[j][:, 512:1024],
                                            wqsn_t[:, m:m + 1], t1[:],
                                            OP.mult, OP.add)

                    # v_ext: transpose vT into per-head [kpos,64]+ones tiles
                    ve = [[], []]
                    with tc.tile_pool(name="veps", bufs=1, space="PSUM") as vps:
                        for t in range(T // 128):
                            tp = vps.tile([128, 128], BF16, name="vtp", bufs=2)
                            nc.tensor.transpose(tp[:],
                                                vT_t[:, 128 * t:128 * (t + 1)],
                                                ident_t[:])
                            for h in range(2):
                                vx = ap.tile([128, 65], BF16, name=f"ve{h}_{t}")
                                nc.vector.tensor_copy(
                                    vx[:, 0:64], tp[:, 64 * h:64 * (h + 1)])
                                nc.vector.memset(vx[:, 64:65], 1.0)
                                ve[h].append(vx)

                    # flash attention (no max subtraction), 2 heads.
                    qp = [qp0, qp1]
                    oTe = [oTe0, oTe1]
                    with tc.tile_pool(name="atw", bufs=1) as aw, \
                         tc.tile_pool(name="atps", bufs=1, space="PSUM") as aps, \
                         tc.tile_pool(name="atpo", bufs=1, space="PSUM") as apo:
                        for qi in range(NCORES):
                            npr = 2 * (qi + 1)
                            for h in range(2):
                                qsl = qp[h][:, 512 * qi:512 * (qi + 1)]
                                ops = apo.tile([65, 512], F32, name="ops",
                                               bufs=2)
                                for p in range(npr):
                                    sp = aps.tile([128, 2, 512], F32,
                                                  name="sp", bufs=3)
                                    for u in range(2):
                                        kb = 2 * p + u
                                        nc.tensor.matmul(
                                            sp[:, u, :],
                                            kT_t[:, 128 * kb:128 * (kb + 1)],
                                            qsl, start=True, stop=True)
                                    est = aw.tile([128, 2, 512], BF16,
                                                  name="est", bufs=4)
                                    nc.scalar.activation(est[:], sp[:],
                                                         AF.Exp, scale=0.125)
                                    if p >= npr - 2:  # diagonal: causal mask
                                        for u in range(2):
                                            jm = 2 * p + u - 4 * qi
                                            nc.vector.tensor_mul(
                                                est[:, u, :], est[:, u, :],
                                                masks_t[:, 512 * jm:512 * (jm + 1)])
                                    for u in range(2):
                                        kb = 2 * p + u
                                        nc.tensor.matmul(
                                            ops[:], ve[h][kb][:],
                                            est[:, u, :],
                                            start=(p == 0 and u == 0),
                                            stop=(p == npr - 1 and u == 1))
                                nc.vector.tensor_copy(
                                    oTe[h][:, 512 * qi:512 * (qi + 1)], ops[:])

                    if DEBUG:
                        nc.sync.dma_start(dbg["d_qp0"][:], qp0[:])
                        nc.sync.dma_start(dbg["d_qp1"][:], qp1[:])
                        nc.sync.dma_start(dbg["d_kT"][:], kT_t[:])
                        nc.sync.dma_start(dbg["d_vT"][:], vT_t[:])
                        nc.sync.dma_start(dbg["d_oTe0"][:], oTe0[:])
                        nc.sync.dma_start(dbg["d_oTe1"][:], oTe1[:])

                    # exchange head outputs + l rows: AllToAll
                    for j in range(NCORES):
                        blk = slice(512 * j, 512 * (j + 1))
                        base = 130 * j
                        nc.sync.dma_start(ob[base:base + 64, :],
                                          oTe0[0:64, blk])
                        nc.sync.dma_start(ob[base + 64:base + 128, :],
                                          oTe1[0:64, blk])
                        nc.sync.dma_start(ob[base + 128:base + 129, :],
                                          oTe0[64:65, blk])
                        nc.sync.dma_start(ob[base + 129:base + 130, :],
                                          oTe1[64:65, blk])
                    nc.gpsimd.collective_compute(
                        "AllToAll", OP.bypass, replica_groups=rg,
                        ins=[ob.opt()], outs=[oax.opt()])

                    # proj: x2T = xT + wproj.T @ (aoutT * 1/l)
                    oax_r = oax.rearrange("(k r) c -> r k c", r=130)
                    with tc.tile_pool(name="prs", bufs=1) as prs, \
                         tc.tile_pool(name="prps", bufs=1, space="PSUM") as pps:
                        wp_t = prs.tile([128, CK, C], BF16, name="wp_t")
                        nc.sync.dma_start(wp_t[:], wproj[:])
                        # all 16 heads' l for own columns -> one 1/l pass
                        l_all = prs.tile([16, 512], BF16, name="l_all")
                        for k in range(CK):
                            nc.sync.dma_start(
                                l_all[2 * k:2 * k + 2, :],
                                oax[130 * k + 128:130 * k + 130, :])
                        lln = prs.tile([16, 512], F32, name="lln")
                        nc.scalar.activation(lln[:], l_all[:], AF.Ln)
                        rl_all = prs.tile([16, 512], BF16, name="rl_all")
                        nc.scalar.activation(rl_all[:], lln[:], AF.Exp,
                                             scale=-1.0)
                        au_all = prs.tile([128, CK, 512], BF16, name="au_all")
                        nc.sync.dma_start(au_all[:], oax_r[0:128])
                        at_tiles = []
                        for k in range(CK):
                            rlb = pps.tile([128, 512], F32, name="rlb", bufs=2)
                            nc.tensor.matmul(
                                rlb[:], sel16_t[:, 128 * k:128 * (k + 1)],
                                rl_all[:], start=True, stop=True)
                            at = prs.tile([128, 512], BF16, name=f"at{k}")
                            nc.vector.tensor_mul(at[:], au_all[:, k, :],
                                                 rlb[:])
                            at_tiles.append(at)
                        for m in range(CK):
                            x2ps = pps.tile([128, 512], F32, name="x2p",
                                            bufs=2)
                            for k in range(CK):
                                nc.tensor.matmul(
                                    x2ps[:], wp_t[:, k, 128 * m:128 * (m + 1)],
                                    at_tiles[k][:], start=(k == 0),
                                    stop=(k == CK - 1))
                            nc.vector.tensor_add(
                                x2T_t[:, m, :], x2ps[:],
                                xT_t[:, m, :].bitcast(F32))

                if DEBUG:
                    nc.sync.dma_start(dbg["d_x2T"][:], x2T_t[:].bitcast(F32))

                # ---------------- MLP scope ----------------
                with tc.tile_pool(name="mlp", bufs=1) as mp:
                    h2T_t = mp.tile([128, CK, 512], BF16, name="h2T_t")
                    _layer_norm(nc, tc, x2T_t, ln2w_t, h2T_t, eps_t,
                                ones_t, ones_b, "2")
                    gel = []
                    with tc.tile_pool(name="fcs", bufs=1) as fs, \
                         tc.tile_pool(name="fcps", bufs=1, space="PSUM") as fps:
                        for g in range(FC // 512):  # 8 groups of 4 m-blocks
                            wg = fs.tile([128, 4, CK, 128], BF16, name="wfcg",
                                         bufs=2)
                            nc.sync.dma_start(wg[:], wfc[:, 4 * g:4 * g + 4])
                            pf = fps.tile([128, 4, 512], F32, name="fcp",
                                          bufs=2)
                            for mm in range(4):
                                for k in range(CK):
                                    nc.tensor.matmul(
                                        pf[:, mm, :], wg[:, mm, k, :],
                                        h2T_t[:, k, :],
                                        start=(k == 0), stop=(k == CK - 1))
                            gl = mp.tile([128, 4, 512], BF16, name=f"gel{g}")
                            nc.scalar.activation(gl[:], pf[:], AF.Gelu)
                            gel.append(gl)
                    # second matmul: single pass, 8 psum accumulators
                    with tc.tile_pool(name="m2s", bufs=1) as m2s, \
                         tc.tile_pool(name="m2ps", bufs=1, space="PSUM") as m2ps:
                        x3ps = [m2ps.tile([128, 512], F32, name=f"x3p{i}")
                                for i in range(CK)]
                        for f4 in range(FC // 512):
                            wm = m2s.tile([128, 4, C], BF16, name="wm",
                                          bufs=2)
                            nc.sync.dma_start(wm[:],
                                              wmlp[:, 4 * f4:4 * f4 + 4, :])
                            for ff in range(4):
                                f = 4 * f4 + ff
                                for i in range(CK):
                                    nc.tensor.matmul(
                                        x3ps[i][:],
                                        wm[:, ff, 128 * i:128 * (i + 1)],
                                        gel[f // 4][:, f % 4, :],
                                        start=(f == 0),
                                        stop=(f == FC // 128 - 1))
                        for i in range(CK):
                            o32 = m2s.tile([128, 512], F32, name="o32",
                                           bufs=2)
                            nc.vector.tensor_add(
                                o32[:], x3ps[i][:],
                                x2T_t[:, i, :].bitcast(F32))
                            nc.sync.dma_start(
                                outT[128 * i:128 * (i + 1), :], o32[:])

    nc.compile()
    return nc


def _host_inputs(x, w_qkv, w_attn_proj, w_fc, w_mlp_proj, ln1_w, ln2_w):
    """Build the 8 per-core input maps."""
    bf = ml_dtypes.bfloat16
    x2 = np.ascontiguousarray(np.asarray(x, np.float32).reshape(T, C))
    w_qkv = np.asarray(w_qkv, np.float32)
    ln1_w = np.asarray(ln1_w, np.float32)
    masks = np.zeros((128, 4 * 512), np.float32)
    kk = np.arange(128)[:, None]
    qq = np.arange(512)[None, :]
    for j in range(4):
        masks[:, 512 * j:512 * (j + 1)] = (qq >= kk + 128 * j)
    masks = masks.astype(bf)
    ident = np.eye(128, dtype=np.float32).astype(bf)
    onesw = np.ones((128, 128), np.float32)
    # sel16[:, 128k + d] = 1 where row r == 2k + d//64 (head of dim d in
    # aout chunk k); broadcasts rl_all rows onto the head-dim rows.
    sel16 = np.zeros((16, CK * 128), np.float32)
    for k in range(CK):
        sel16[2 * k, 128 * k:128 * k + 64] = 1.0
        sel16[2 * k + 1, 128 * k + 64:128 * (k + 1)] = 1.0
    sel16 = sel16.astype(bf)
    ln2 = np.ascontiguousarray(np.asarray(ln2_w, np.float32).reshape(CK, 128).T)
    # full x, transposed + C-chunked, bf16: xb[p, k, t] = x[t, 128k+p]
    xball = np.ascontiguousarray(
        x2.T.reshape(CK, 128, T).transpose(1, 0, 2).astype(bf))
    wproj = np.asarray(w_attn_proj, np.float32).reshape(CK, 128, C) \
        .transpose(1, 0, 2).astype(bf)
    wfc = np.asarray(w_fc, np.float32).reshape(CK, 128, FC // 128, 128) \
        .transpose(1, 2, 0, 3).astype(bf)
    wmlp = np.asarray(w_mlp_proj, np.float32).reshape(FC // 128, 128, C) \
        .transpose(1, 0, 2).astype(bf)
    common = {
        "xb": xball,
        "wproj": np.ascontiguousarray(wproj),
        "wfc": np.ascontiguousarray(wfc),
        "wmlp": np.ascontiguousarray(wmlp),
        "ln2w": ln2, "masks": masks, "ident": ident,
        "onesw": onesw, "sel16": sel16,
        "zeros": np.zeros((64, T), bf),
    }
    in_maps = []
    for c in range(NCORES):
        xTc = np.ascontiguousarray(x2[TC * c:TC * (c + 1), :].T)
        wq = np.concatenate(
            [w_qkv[:, C * s + 128 * c:C * s + 128 * (c + 1)] for s in range(3)],
            axis=1)  # [C, 384] pre-folded with ln1 weight
        wq = wq * ln1_w[:, None]
        wqs = wq.sum(axis=0)  # [384]
        wqsn = np.ascontiguousarray(-wqs.reshape(3, 128).T.astype(np.float32))
        wq = np.ascontiguousarray(
            wq.reshape(CK, 128, 3 * 128).transpose(1, 0, 2).astype(bf))
        in_maps.append({"xT": xTc, "wqkv": wq, "wqsn": wqsn, **common})
    return in_maps


def _run(in_maps, **kw):
    key = ("nc", DEBUG)
    if key not in _CACHE:
        _CACHE[key] = _build()
    return bass_utils.run_bass_kernel_spmd(
        _CACHE[key], in_maps, core_ids=list(range(NCORES)), **kw)


def kernel(x, w_qkv, w_attn_proj, w_fc, w_mlp_proj, ln1_w, ln2_w):
    in_maps = _host_inputs(x, w_qkv, w_attn_proj, w_fc, w_mlp_proj,
                           ln1_w, ln2_w)
    res = _run(in_maps)
    out = np.empty((1, T, C), np.float32)
    for c in range(NCORES):
        out[0, TC * c:TC * (c + 1), :] = res.results[c]["outT"].T
    return out
